# revision 28
# baseline (speedup 1.0000x reference)
"""GAT spatio-temporal model Trainium2 kernel (v6).

Sharding: data-parallel over batch B=8 -> 8 NeuronCores (1 graph each).

Attention factorization (exact): with E = exp(s), Ea = exp(alpha*s),
exp(lrelu(s1[n]+s2[m])) = max(E1[n]E2[m], E1a[n]E2a[m]).  Dividing by
E1a[n] (constant along the softmax axis, cancels):
    p[m,n] = max(E2[m]*E1b[n], E2a[m]) * mask[n,m],  E1b = exp(beta*s1)
so E2 is folded INTO the score tensor (v6):
 - t = tensor_scalar(e1b_bcast, *E2[m], max E2a[m]) -- 2x DVE mode
 - s_t = t * maskT (one batched [128,4N] tensor_tensor per head)
 - num = sum_m projN_plain[m,F] s_t[m,n]  (plain batched PSUM evacs)
 - den = sum_m s_t[m,n] via zero-padded ones lhsT, 4 heads -> one [4,N]
   PSUM tile -> ONE reciprocal + cast per group.
 - 1/den and E1b_o broadcasts via PE rank-1 + ACT evac (low latency).
 - h2N via PE transposes of h2_bf (not 32 small matmuls).
 - LN: stats for the 4 n-chunks land on 4 PSUM partitions (zero-padded
   1/F lhsT) so row ops run 4x fewer elements; affine+ReLU fused into
   one ACT (scale=g, bias=b per partition).

Shapes (hardcoded): B=8, N=512, Din=64, H=8, F=128, L=2.
"""
import os
import numpy as np
from contextlib import ExitStack

import concourse.bass as bass
import concourse.tile as tile
from concourse import bacc, mybir
from concourse.bass_utils import run_bass_kernel_spmd
from concourse.masks import make_identity

F32 = mybir.dt.float32
BF16 = mybir.dt.bfloat16
AF = mybir.ActivationFunctionType
OP = mybir.AluOpType

B, N, DIN, H, F, L = 8, 512, 64, 8, 128, 2
NCHUNK = N // 128  # 4
NG = 2             # den groups per layer (4 heads each)
GH = H // NG       # heads per group
ALPHA = 0.2
BETA = 1.0 - ALPHA
LN_EPS = 1e-5

_CACHE = {}


def build_nc():
    nc = bacc.Bacc("TRN2", target_bir_lowering=False, debug=False)

    x_d = nc.dram_tensor("x", [N, DIN], F32, kind="ExternalInput").ap()
    adj_d = nc.dram_tensor("adj", [N, N], mybir.dt.int32, kind="ExternalInput").ap()
    Wp_d = nc.dram_tensor("Wp", [DIN, F], F32, kind="ExternalInput").ap()
    bp_d = nc.dram_tensor("bp", [F], F32, kind="ExternalInput").ap()
    Wh_d = nc.dram_tensor("W_heads", [L, H, F, F], F32, kind="ExternalInput").ap()
    ah_d = nc.dram_tensor("a_heads", [L, H, 2 * F], F32, kind="ExternalInput").ap()
    Wo_d = nc.dram_tensor("W_out", [L, H * F, F], F32, kind="ExternalInput").ap()
    ao_d = nc.dram_tensor("a_out", [L, 2 * F], F32, kind="ExternalInput").ap()
    g_d = nc.dram_tensor("ln_g", [L, F], F32, kind="ExternalInput").ap()
    b_d = nc.dram_tensor("ln_b", [L, F], F32, kind="ExternalInput").ap()
    out_d = nc.dram_tensor("out", [N, F], F32, kind="ExternalOutput").ap()
    # DRAM bounce buffers for the E1b and 1/den row broadcasts
    ebl_d = [nc.dram_tensor(f"eblk{l}", [16, N], BF16, kind="ExternalOutput").ap()
             for l in range(L)]
    rrd_d = [nc.dram_tensor(f"rrd{l}", [GH, N], BF16, kind="ExternalOutput").ap()
             for l in range(L)]

    with tile.TileContext(nc) as tc, ExitStack() as ctx:
        const = ctx.enter_context(tc.tile_pool(name="const", bufs=1))
        sx = ctx.enter_context(tc.tile_pool(name="sx", bufs=2))
        sproj = ctx.enter_context(tc.tile_pool(name="sproj", bufs=2))
        sbcast = ctx.enter_context(tc.tile_pool(name="sbcast", bufs=9))
        sexp = ctx.enter_context(tc.tile_pool(name="sexp", bufs=7))
        smulti = ctx.enter_context(tc.tile_pool(name="smulti", bufs=9))
        sbig = ctx.enter_context(tc.tile_pool(name="sbig", bufs=3))
        srow = ctx.enter_context(tc.tile_pool(name="srow", bufs=2))
        shd = ctx.enter_context(tc.tile_pool(name="shd", bufs=4))
        smask = ctx.enter_context(tc.tile_pool(name="smask", bufs=4))
        pou = ctx.enter_context(tc.tile_pool(name="pou", bufs=3, space="PSUM"))
        pmisc = ctx.enter_context(tc.tile_pool(name="pmisc", bufs=2, space="PSUM"))
        prow = ctx.enter_context(tc.tile_pool(name="prow", bufs=2, space="PSUM"))

        # ---------------- input DMAs first, spread across all hw queues ----
        x_chunks = []
        for c in range(NCHUNK):
            xc = shd.tile([128, DIN], F32, tag="xchunk")
            nc.sync.dma_start(xc, x_d[bass.ts(c, 128), :])
            x_chunks.append(xc)
        Wh_ball = [const.tile([F, H, F], BF16, name=f"WhB{l}") for l in range(L)]
        Wh0_f = const.tile([F, H, F], F32)
        nc.scalar.dma_start(Wh0_f, Wh_d[0].rearrange("h i o -> i h o"))
        adj_qs = [nc.sync, nc.scalar, nc.sync, nc.scalar]
        adj_raw = []
        for r in range(NCHUNK):
            ai = shd.tile([128, N], mybir.dt.int32, tag="adji", bufs=4)
            adj_qs[r].dma_start(ai, adj_d[bass.ts(r, 128), :])
            adj_raw.append(ai)
        bp_col = const.tile([F, 1], F32)
        nc.sync.dma_start(bp_col, bp_d.rearrange("(f one) -> f one", one=1))

        # ---------------- constants (before gpsimd queue work) ----------------
        ones_row_bf = const.tile([1, N], BF16)
        nc.vector.memset(ones_row_bf, 1.0)
        ones_col_bf = const.tile([128, 1], BF16)
        nc.vector.memset(ones_col_bf, 1.0)
        ident = const.tile([128, 128], F32)
        make_identity(nc, ident)
        ident_bf = const.tile([128, 128], BF16)
        nc.vector.tensor_copy(ident_bf, ident)
        eps_col = const.tile([128, 1], F32)
        nc.vector.memset(eps_col, LN_EPS)
        # onespad[:, j, k] = 1 iff k == j  (den-group lhsT)
        onespad = const.tile([128, GH, GH], BF16)
        nc.vector.memset(onespad, 0.0)
        for j in range(GH):
            nc.vector.memset(onespad[:, j, j:j + 1], 1.0)
        # invFpad[:, c, k] = 1/F iff k == c  (LN 4-partition stats lhsT)
        invFpad = const.tile([128, NCHUNK, NCHUNK], BF16)
        nc.vector.memset(invFpad, 0.0)
        for c in range(NCHUNK):
            nc.vector.memset(invFpad[:, c, c:c + 1], 1.0 / F)
        # sel4[k, j, :] = 1 iff k == j: row-selector lhsT for rank-1
        # broadcasts out of [4, N] tiles (rhs base partition must be 0)
        sel4 = const.tile([4, NCHUNK, 128], BF16)
        nc.gpsimd.memset(sel4, 0.0)
        nc.gpsimd.affine_select(
            out=sel4, in_=sel4, compare_op=OP.not_equal, fill=1.0,
            base=0, pattern=[[-1, NCHUNK], [0, 128]], channel_multiplier=1)
        # sel16[k, h, :] = 1 iff k == 2h: selects the E1b rows of Eblk
        sel16 = const.tile([16, 2, 128], BF16)
        nc.gpsimd.memset(sel16, 0.0)
        nc.gpsimd.affine_select(
            out=sel16, in_=sel16, compare_op=OP.not_equal, fill=1.0,
            base=0, pattern=[[-2, 2], [0, 128]], channel_multiplier=1)

        # gpsimd software-queue weight loads (after the const builds so the
        # identity/selector are ready early for the PE transposes)
        Wp_sb = const.tile([DIN, F], BF16)
        nc.gpsimd.dma_start(Wp_sb, Wp_d)
        ah_ball = const.tile([F, L * H, 2], BF16)
        nc.gpsimd.dma_start(ah_ball, ah_d.rearrange("l h (t f) -> f (l h) t", t=2))
        ah_bf = [[ah_ball[:, l * H + h, :] for h in range(H)] for l in range(L)]
        ao_ball = const.tile([F, L, 2], BF16)
        nc.gpsimd.dma_start(ao_ball, ao_d.rearrange("l (t f) -> f l t", t=2))
        ao_bf = [ao_ball[:, l, :] for l in range(L)]
        gb_all = const.tile([F, 2 * L], F32)
        nc.gpsimd.dma_start(gb_all[:, 0:L], g_d.rearrange("l f -> f l"))
        nc.gpsimd.dma_start(gb_all[:, L:2 * L], b_d.rearrange("l f -> f l"))
        g_col = [gb_all[:, l:l + 1] for l in range(L)]
        b_col = [gb_all[:, L + l:L + l + 1] for l in range(L)]
        Wo_ball = [const.tile([128, H, F], BF16, name=f"WoB{l}") for l in range(L)]
        nc.gpsimd.dma_start(Wo_ball[0], Wo_d[0].rearrange("(c p) f -> p c f", p=128))
        nc.gpsimd.dma_start(Wh_ball[1], Wh_d[1].rearrange("h i o -> i h o"))
        nc.gpsimd.dma_start(Wo_ball[1], Wo_d[1].rearrange("(c p) f -> p c f", p=128))
        Wo_bf = Wo_ball

        Wh_bf = [[Wh_ball[l][:, h, :] for h in range(H)] for l in range(L)]

        # ------------- per-layer weight prep: WhT, Wtilde, WoT, aoWo -------
        WhT_ball = [const.tile([F, H, F], BF16, name=f"WhT{l}") for l in range(L)]
        WoT_ball = [const.tile([F, H, F], BF16, name=f"WoT{l}") for l in range(L)]
        aoWo_ball = const.tile([F, L * H, 2], BF16)
        aoWo = [[aoWo_ball[:, l * H + h, :] for h in range(H)] for l in range(L)]
        Wt_bf = [const.tile([F, 2 * H], BF16, name=f"Wt{l}") for l in range(L)]

        def prep_attn_weights(l):
            if l == 0:
                nc.vector.tensor_copy(Wh_ball[0], Wh0_f)
            for h in range(H):
                pt = pou.tile([128, 128], BF16, tag="oU")
                nc.tensor.transpose(pt, Wh_bf[l][h], ident_bf)
                # layer-0 prep runs pre-dense: keep ACT free for the
                # Eblk/e1b/C_e2 chain that gates the first score op
                if l > 0 and h % 2 == 0:
                    nc.scalar.activation(WhT_ball[l][:, h, :], pt, AF.Copy)
                else:
                    nc.vector.tensor_copy(WhT_ball[l][:, h, :], pt)
            pw = prow.tile([128, 2 * H], F32, tag="prow")
            for h in range(H):
                nc.tensor.matmul(pw[:, 2 * h:2 * h + 2], WhT_ball[l][:, h, :],
                                 ah_bf[l][h], start=True, stop=True)
            nc.scalar.activation(Wt_bf[l], pw, AF.Copy)

        def prep_out_weights(l):
            for h in range(H):
                pt = pou.tile([128, 128], BF16, tag="oU")
                nc.tensor.transpose(pt, Wo_ball[l][:, h, :], ident_bf)
                if h % 2 == 0:
                    nc.scalar.activation(WoT_ball[l][:, h, :], pt, AF.Copy)
                else:
                    nc.vector.tensor_copy(WoT_ball[l][:, h, :], pt)
            paw = prow.tile([128, 2 * H], F32, tag="prow")
            for h in range(H):
                nc.tensor.matmul(paw[:, 2 * h:2 * h + 2],
                                 WoT_ball[l][:, h, :], ao_bf[l],
                                 start=True, stop=True)
            nc.scalar.activation(
                aoWo_ball[:, l * H:(l + 1) * H, :].rearrange("p h t -> p (h t)"),
                paw, AF.Copy)

        # ---------------- x -> xT, input projection ----------------
        xT = const.tile([DIN, N], BF16)
        ph = pmisc.tile([128, N], F32, tag="pbig")
        hT_bf = sbig.tile([128, N], BF16, tag="hTb", bufs=2)
        for c in range(NCHUNK):
            xb = shd.tile([128, DIN], BF16, tag="xchb")
            nc.vector.tensor_copy(xb, x_chunks[c])
            pt = pmisc.tile([DIN, 128], BF16, tag="pbig")
            nc.tensor.transpose(pt, xb, ident_bf)
            nc.scalar.activation(xT[:, bass.ts(c, 128)], pt, AF.Copy)
            nc.tensor.matmul(ph[:, bass.ts(c, 128)], Wp_sb, xT[:, bass.ts(c, 128)],
                             start=True, stop=True)
            nc.scalar.activation(hT_bf[:, bass.ts(c, 128)], ph[:, bass.ts(c, 128)],
                                 AF.Relu, bias=bp_col)
        hT = hT_bf

        prep_attn_weights(0)

        # ---------------- adj -> maskT (bf16, transposed) ----------------
        adj_f = []
        for r in range(NCHUNK):
            af = smask.tile([128, N], BF16, tag="adjf")
            nc.vector.tensor_copy(af, adj_raw[r])
            adj_f.append(af)
        maskT_all = const.tile([128, NCHUNK, N], BF16)
        maskT = [maskT_all[:, c, :] for c in range(NCHUNK)]
        for r in range(NCHUNK):
            for c in range(NCHUNK):
                pm = pmisc.tile([128, 128], BF16, tag="pbig")
                nc.tensor.transpose(pm, adj_f[r][:, bass.ts(c, 128)], ident_bf)
                nc.vector.tensor_copy(maskT[c][:, bass.ts(r, 128)], pm)

        # ---------------- layers ----------------
        for l in range(L):
            residT = hT
            # --- rows for all heads: s12[2h] = s1_h, s12[2h+1] = s2_h
            s12_ps = prow.tile([2 * H, N], F32, tag="prow")
            nc.tensor.matmul(s12_ps, Wt_bf[l], hT_bf, start=True, stop=True)
            Eblk = sx.tile([16, N], BF16, tag="Eblk")   # exp(+beta*s): rows 2h = E1b
            nc.scalar.activation(Eblk, s12_ps, AF.Exp, scale=BETA)
            # E1b broadcasts: one DRAM bounce write of all rows, then one
            # stride-0 broadcast read per head, spread across DMA queues
            dmaq = [nc.sync, nc.gpsimd]
            nc.sync.dma_start(ebl_d[l], Eblk)
            e1b = []
            for h in range(H):
                eb = sbcast.tile([128, N], BF16, tag="e1b", bufs=9)
                if h < 2:
                    # first heads via PE rank-1 + ACT evac: available ~1.5us
                    # earlier than the DRAM bounce, starts the dense phase
                    ebp = pmisc.tile([128, N], F32, tag="pbig")
                    nc.tensor.matmul(ebp, sel16[:, h, :], Eblk,
                                     start=True, stop=True)
                    nc.scalar.activation(eb, ebp, AF.Copy)
                else:
                    row = ebl_d[l][2 * h, :]
                    src_bc = bass.AP(tensor=row.tensor, offset=row.offset,
                                     ap=[[0, 128], [1, N]])
                    dmaq[h % 2].dma_start(eb, src_bc)
                e1b.append(eb)
            # --- s2 columns directly via tiny matmuls (no transposes)
            Wt2 = Wt_bf[l].rearrange("i (h t) -> i t h", t=2)[:, 1, :]
            cps = prow.tile([128, NCHUNK, 8], F32, tag="prow")
            for c in range(NCHUNK):
                nc.tensor.matmul(cps[:, c, :], hT_bf[:, bass.ts(c, 128)], Wt2,
                                 start=True, stop=True)
            C_e2f = sx.tile([128, NCHUNK, 8], F32, tag="Ce2f")
            nc.scalar.activation(C_e2f, cps, AF.Exp, scale=1.0)
            C_e2a = sx.tile([128, NCHUNK, 8], F32, tag="Ce2a")
            nc.scalar.activation(C_e2a, cps, AF.Exp, scale=ALPHA)

            def e2_col(h, c):
                return C_e2f[:, c, h:h + 1]

            def e2a_col(h, c):
                return C_e2a[:, c, h:h + 1]

            # --- projN: batched over heads (2 x 512-free MMs per chunk),
            # plain batched evacuation (E2 lives in s_t now)
            projAll = sproj.tile([128, NCHUNK, H * 128], BF16, tag="projAll",
                                 name=f"pa{l}", bufs=1)
            WhV = Wh_ball[l].rearrange("i h f -> i (h f)")
            for c in range(NCHUNK):
                for g in range(2):
                    pN = pmisc.tile([128, N], F32, tag="pbig")
                    nc.tensor.matmul(pN, hT_bf[:, bass.ts(c, 128)],
                                     WhV[:, bass.ts(g, 512)], start=True, stop=True)
                    dst = projAll[:, c, g * 512:(g + 1) * 512]
                    nc.scalar.activation(dst, pN, AF.Copy)

            def proj_ct(h, c):
                return projAll[:, c, h * 128:(h + 1) * 128]

            if l == 0:
                # deferred weight prep runs inside layer-0's dense phase
                prep_out_weights(0)
                prep_attn_weights(1)
                prep_out_weights(1)

            # --- attention per group of GH heads; pou/rep land in PAIR
            # tiles (2 heads) so normalize+ELU run as 2-head-wide ops:
            # bigger DVE/ACT ops, only a ~0.4us pair barrier
            pobp = [None] * (H // 2)
            repp = [None] * (H // 2)
            for g in range(NG):
                deng_ps = prow.tile([GH, N], F32, tag="deng", bufs=1)
                for j in range(GH):
                    h = g * GH + j
                    if h % 2 == 0:
                        pobp[h // 2] = smulti.tile([128, 2, N], BF16,
                                                   tag="pob", bufs=4,
                                                   name=f"pobp{l}_{h}")
                    tten = sexp.tile([128, NCHUNK, N], BF16, tag="tten", bufs=3)
                    for c in range(NCHUNK):
                        nc.vector.tensor_scalar(tten[:, c, :], e1b[h],
                                                e2_col(h, c), e2a_col(h, c),
                                                OP.mult, OP.max)
                    s_t = sexp.tile([128, NCHUNK, N], BF16, tag="s_t")
                    nc.vector.tensor_tensor(s_t, tten, maskT_all, OP.mult)
                    for c in range(NCHUNK):
                        nc.tensor.matmul(deng_ps, onespad[:, j, :], s_t[:, c, :],
                                         start=(j == 0 and c == 0),
                                         stop=(j == GH - 1 and c == NCHUNK - 1))
                    pou_ps = pou.tile([128, N], F32, tag="oU")
                    for c in range(NCHUNK):
                        nc.tensor.matmul(pou_ps, proj_ct(h, c), s_t[:, c, :],
                                         start=(c == 0), stop=(c == NCHUNK - 1))
                    nc.scalar.activation(pobp[h // 2][:, h % 2, :], pou_ps,
                                         AF.Copy)
                rr4 = srow.tile([GH, N], F32, tag="rr4")
                nc.vector.reciprocal_approx_fast(rr4, deng_ps)
                rr4_bf = srow.tile([GH, N], BF16, tag="rr4b")
                nc.vector.tensor_copy(rr4_bf, rr4)
                for j in range(GH):
                    h = g * GH + j
                    if h % 2 == 0:
                        repp[h // 2] = sbcast.tile([128, 2, N], BF16,
                                                   tag="rep", bufs=3,
                                                   name=f"repp{l}_{h}")
                    rep_ps = pmisc.tile([128, N], F32, tag="pbig")
                    nc.tensor.matmul(rep_ps, sel4[:, j, :], rr4_bf,
                                     start=True, stop=True)
                    nc.scalar.activation(repp[h // 2][:, h % 2, :], rep_ps,
                                         AF.Copy)

            # --- normalize + ELU, 2-head-wide
            multiT = []
            for p in range(H // 2):
                outp = sbig.tile([128, 2, N], BF16, tag="outT", bufs=2)
                nc.vector.tensor_tensor(outp, pobp[p], repp[p], OP.mult)
                exp_ = shd.tile([128, 2, N], BF16, tag="elu_ex", bufs=2)
                nc.scalar.activation(exp_, outp, AF.Exp)
                ex2p = shd.tile([128, 2, N], BF16, tag="elu_ex2", bufs=2)
                nc.vector.tensor_scalar(ex2p, exp_, 1.0, -1.0, OP.min, OP.add)
                mq = smulti.tile([128, 2, N], BF16, tag="multi", bufs=4)
                nc.vector.tensor_tensor(mq, outp, ex2p, OP.max)
                multiT.append(mq[:, 0, :])
                multiT.append(mq[:, 1, :])

            # --- W_out projection (h2 in F-layout)
            ph2 = pou.tile([128, N], F32, tag="oU")
            for h in range(H):
                nc.tensor.matmul(ph2, Wo_bf[l][:, h, :], multiT[h],
                                 start=(h == 0), stop=(h == H - 1))
            h2_bf = sbig.tile([128, N], BF16, tag="h2b", bufs=2)
            nc.scalar.activation(h2_bf, ph2, AF.Copy)

            # --- single out-attention: s12o accumulated per head from multiT
            # via aoWo (no wait on the full h2_bf)
            s12o_ps = prow.tile([2, N], F32, tag="prow")
            for h in range(H):
                nc.tensor.matmul(s12o_ps, aoWo[l][h], multiT[h],
                                 start=(h == 0), stop=(h == H - 1))
            Xo_b = sx.tile([1, N], BF16, tag="Xo_b")    # E1b_o row
            nc.scalar.activation(Xo_b, s12o_ps[0:1, :], AF.Exp, scale=BETA)
            ebo_ps = pmisc.tile([128, N], F32, tag="pbig")
            nc.tensor.matmul(ebo_ps, ones_row_bf[:, 0:128], Xo_b,
                             start=True, stop=True)
            e1bo = sbcast.tile([128, N], BF16, tag="e1b", bufs=9)
            nc.scalar.activation(e1bo, ebo_ps, AF.Copy)
            so_ps = prow.tile([128, NCHUNK, 2], F32, tag="prow")
            for c in range(NCHUNK):
                nc.tensor.matmul(so_ps[:, c, :], h2_bf[:, bass.ts(c, 128)],
                                 ao_bf[l], start=True, stop=True)
            Co_e2f = sx.tile([128, NCHUNK, 2], F32, tag="Coe2f")
            nc.scalar.activation(Co_e2f, so_ps, AF.Exp, scale=1.0)
            Co_e2a = sx.tile([128, NCHUNK, 2], F32, tag="Coe2a")
            nc.scalar.activation(Co_e2a, so_ps, AF.Exp, scale=ALPHA)
            # h2N via PE transposes of h2_bf, plain evac (E2_o lives in s_to)
            h2Np = sproj.tile([128, NCHUNK, 128], BF16, tag="h2Np")
            for c in range(NCHUNK):
                pm = pmisc.tile([128, 128], BF16, tag="pbig")
                nc.tensor.transpose(pm, h2_bf[:, bass.ts(c, 128)], ident_bf)
                nc.scalar.activation(h2Np[:, c, :], pm, AF.Copy)
            # out-att scores
            tto = sexp.tile([128, NCHUNK, N], BF16, tag="tten", bufs=3)
            for c in range(NCHUNK):
                nc.vector.tensor_scalar(tto[:, c, :], e1bo,
                                        Co_e2f[:, c, 1:2], Co_e2a[:, c, 1:2],
                                        OP.mult, OP.max)
            s_to = sexp.tile([128, NCHUNK, N], BF16, tag="s_t")
            nc.vector.tensor_tensor(s_to[:, 0:2, :], tto[:, 0:2, :],
                                    maskT_all[:, 0:2, :], OP.mult)
            nc.vector.tensor_tensor(s_to[:, 2:4, :], tto[:, 2:4, :],
                                    maskT_all[:, 2:4, :], OP.mult)
            deno_ps = prow.tile([1, N], F32, tag="prow")
            for c in range(NCHUNK):
                nc.tensor.matmul(deno_ps, ones_col_bf, s_to[:, c, :],
                                 start=(c == 0), stop=(c == NCHUNK - 1))
            pouo_ps = pou.tile([128, N], F32, tag="oU")
            for c in range(NCHUNK):
                nc.tensor.matmul(pouo_ps, h2Np[:, c, :], s_to[:, c, :],
                                 start=(c == 0), stop=(c == NCHUNK - 1))
            rro = srow.tile([1, N], F32, tag="rro")
            nc.vector.reciprocal_approx_fast(rro, deno_ps)
            rro_bf = srow.tile([1, N], BF16, tag="rrob")
            nc.vector.tensor_copy(rro_bf, rro)
            rpo_ps = pmisc.tile([128, N], F32, tag="pbig")
            nc.tensor.matmul(rpo_ps, ones_row_bf[:, 0:128], rro_bf,
                             start=True, stop=True)
            pobo = smulti.tile([128, N], BF16, tag="pobo", bufs=2)
            nc.scalar.activation(pobo, pouo_ps, AF.Copy)
            outsT = sbig.tile([128, N], BF16, tag="outsT", bufs=2)
            nc.vector.tensor_tensor(outsT, pobo, rpo_ps, OP.mult)

            # ---- residual + LN over partition dim (bf16 stream) ----
            xs = sbig.tile([128, N], BF16, tag="xs", bufs=2)
            nc.vector.tensor_tensor(xs, outsT, residT, OP.add)
            xsq = sbig.tile([128, N], BF16, tag="xsq", bufs=2)
            nc.scalar.activation(xsq, xs, AF.Square)
            # 4-partition stats: row c of [4,128] = mean/meansq of chunk c
            pmu4 = prow.tile([NCHUNK, 128], F32, tag="prow")
            for c in range(NCHUNK):
                nc.tensor.matmul(pmu4, invFpad[:, c, :], xs[:, bass.ts(c, 128)],
                                 start=(c == 0), stop=(c == NCHUNK - 1))
            psq4 = prow.tile([NCHUNK, 128], F32, tag="prow")
            for c in range(NCHUNK):
                nc.tensor.matmul(psq4, invFpad[:, c, :], xsq[:, bass.ts(c, 128)],
                                 start=(c == 0), stop=(c == NCHUNK - 1))
            mu2 = srow.tile([NCHUNK, 128], F32, tag="rowL", bufs=4)
            nc.scalar.activation(mu2, pmu4, AF.Square)
            nmu4_bf = srow.tile([NCHUNK, 128], BF16, tag="rowLb", bufs=2)
            nc.vector.tensor_scalar_mul(nmu4_bf, pmu4, -1.0)
            # rstd = (var+eps)^-1/2 via int32-view seed + 2 Newton steps
            # (avoids Ln/Exp -> no ACT table switching)
            ve = srow.tile([NCHUNK, 128], F32, tag="rowL", bufs=4)
            nc.vector.scalar_tensor_tensor(ve, psq4, LN_EPS, mu2,
                                           OP.add, OP.subtract)
            y0 = srow.tile([NCHUNK, 128], F32, tag="rowL", bufs=4)
            nc.vector.tensor_scalar(y0.bitcast(mybir.dt.int32),
                                    ve.bitcast(mybir.dt.int32),
                                    -0.5, 1597463007.0, OP.mult, OP.add)
            w1 = srow.tile([NCHUNK, 128], F32, tag="rowL", bufs=4)
            nc.vector.tensor_tensor(w1, y0, y0, OP.mult)
            z1 = srow.tile([NCHUNK, 128], F32, tag="rowL", bufs=4)
            nc.vector.scalar_tensor_tensor(z1, ve, -0.5, w1, OP.mult, OP.mult)
            y1 = srow.tile([NCHUNK, 128], F32, tag="rowL", bufs=4)
            nc.vector.scalar_tensor_tensor(y1, z1, 1.5, y0, OP.add, OP.mult)
            w2 = srow.tile([NCHUNK, 128], F32, tag="rowL", bufs=4)
            nc.vector.tensor_tensor(w2, y1, y1, OP.mult)
            z2 = srow.tile([NCHUNK, 128], F32, tag="rowL", bufs=4)
            nc.vector.scalar_tensor_tensor(z2, ve, -0.5, w2, OP.mult, OP.mult)
            rstd4_bf = srow.tile([NCHUNK, 128], BF16, tag="rowLb", bufs=2)
            nc.vector.scalar_tensor_tensor(rstd4_bf, z2, 1.5, y1,
                                           OP.add, OP.mult)
            # (2 Newton steps: rstd to ~1e-5 rel; plenty for the 2e-2 gate)
            rep_rstd = pmisc.tile([128, N], F32, tag="pbig")
            rep_nmu = pmisc.tile([128, N], F32, tag="pbig")
            for c in range(NCHUNK):
                nc.tensor.matmul(rep_rstd[:, bass.ts(c, 128)],
                                 sel4[:, c, :], rstd4_bf,
                                 start=True, stop=True)
                nc.tensor.matmul(rep_nmu[:, bass.ts(c, 128)],
                                 sel4[:, c, :], nmu4_bf,
                                 start=True, stop=True)
            rep_nmu_b = sbcast.tile([128, N], BF16, tag="repl", bufs=2)
            nc.scalar.activation(rep_nmu_b, rep_nmu, AF.Copy)
            rep_rstd_b = sbcast.tile([128, N], BF16, tag="repl", bufs=2)
            nc.scalar.activation(rep_rstd_b, rep_rstd, AF.Copy)
            u = sbig.tile([128, N], BF16, tag="u", bufs=2)
            nc.vector.tensor_tensor(u, xs, rep_nmu_b, OP.add)
            t2 = sbig.tile([128, N], BF16, tag="t2", bufs=2)
            nc.vector.tensor_tensor(t2, u, rep_rstd_b, OP.mult)
            hT_bf = sbig.tile([128, N], BF16, tag="hTb", bufs=2)
            fn = AF.Relu if l < L - 1 else AF.Identity
            nc.scalar.activation(hT_bf, t2, fn, scale=g_col[l], bias=b_col[l])
            hT = hT_bf

        # ---------------- output: transpose back ----------------
        for c in range(NCHUNK):
            po = pmisc.tile([128, 128], BF16, tag="pbig")
            nc.tensor.transpose(po, hT[:, bass.ts(c, 128)], ident_bf)
            osb = shd.tile([128, 128], F32, tag="osb")
            nc.scalar.activation(osb, po, AF.Copy)
            nc.sync.dma_start(out_d[bass.ts(c, 128), :], osb)

    nc.compile()
    return nc


def _get_nc():
    if "nc" not in _CACHE:
        _CACHE["nc"] = build_nc()
    return _CACHE["nc"]


def kernel(**inputs) -> np.ndarray:
    nc = _get_nc()
    shared = {k: np.ascontiguousarray(np.asarray(inputs[k], dtype=np.float32))
              for k in ("Wp", "bp", "W_heads", "a_heads", "W_out", "a_out",
                        "ln_g", "ln_b")}
    x = np.asarray(inputs["x"], dtype=np.float32)
    adj = np.asarray(inputs["adj"], dtype=np.int32)
    in_maps = [dict(x=np.ascontiguousarray(x[b]),
                    adj=np.ascontiguousarray(adj[b]), **shared)
               for b in range(B)]
    res = run_bass_kernel_spmd(nc, in_maps, core_ids=list(range(B)))
    return np.stack([res.results[b]["out"] for b in range(B)])


if __name__ == "__main__":
    rng = np.random.default_rng(0)
    inputs = dict(
        x=rng.normal(size=(B, N, DIN)).astype(np.float32),
        adj=rng.integers(0, 2, size=(B, N, N)).astype(np.int32),
        Wp=(rng.normal(size=(DIN, F)) * 0.12).astype(np.float32),
        bp=np.zeros(F, dtype=np.float32),
        W_heads=(rng.normal(size=(L, H, F, F)) * 0.08).astype(np.float32),
        a_heads=(rng.normal(size=(L, H, 2 * F)) * 0.08).astype(np.float32),
        W_out=(rng.normal(size=(L, H * F, F)) * 0.03).astype(np.float32),
        a_out=(rng.normal(size=(L, 2 * F)) * 0.08).astype(np.float32),
        ln_g=np.ones((L, F), dtype=np.float32),
        ln_b=np.zeros((L, F), dtype=np.float32),
    )
    out = kernel(**inputs)
    print("out", out.shape, out.dtype, np.abs(out).max())


# revision 29
# speedup vs baseline: 1.0206x; 1.0206x over previous
"""GAT spatio-temporal model Trainium2 kernel (v6).

Sharding: data-parallel over batch B=8 -> 8 NeuronCores (1 graph each).

Attention factorization (exact): with E = exp(s), Ea = exp(alpha*s),
exp(lrelu(s1[n]+s2[m])) = max(E1[n]E2[m], E1a[n]E2a[m]).  Dividing by
E1a[n] (constant along the softmax axis, cancels):
    p[m,n] = max(E2[m]*E1b[n], E2a[m]) * mask[n,m],  E1b = exp(beta*s1)
so E2 is folded INTO the score tensor (v6):
 - t = tensor_scalar(e1b_bcast, *E2[m], max E2a[m]) -- 2x DVE mode
 - s_t = t * maskT (one batched [128,4N] tensor_tensor per head)
 - num = sum_m projN_plain[m,F] s_t[m,n]  (plain batched PSUM evacs)
 - den = sum_m s_t[m,n] via zero-padded ones lhsT, 4 heads -> one [4,N]
   PSUM tile -> ONE reciprocal + cast per group.
 - 1/den and E1b_o broadcasts via PE rank-1 + ACT evac (low latency).
 - h2N via PE transposes of h2_bf (not 32 small matmuls).
 - LN: stats for the 4 n-chunks land on 4 PSUM partitions (zero-padded
   1/F lhsT) so row ops run 4x fewer elements; affine+ReLU fused into
   one ACT (scale=g, bias=b per partition).

Shapes (hardcoded): B=8, N=512, Din=64, H=8, F=128, L=2.
"""
import os
import numpy as np
from contextlib import ExitStack

import concourse.bass as bass
import concourse.tile as tile
from concourse import bacc, mybir
from concourse.bass_utils import run_bass_kernel_spmd
from concourse.masks import make_identity

F32 = mybir.dt.float32
BF16 = mybir.dt.bfloat16
AF = mybir.ActivationFunctionType
OP = mybir.AluOpType

B, N, DIN, H, F, L = 8, 512, 64, 8, 128, 2
NCHUNK = N // 128  # 4
NG = 2             # den groups per layer (4 heads each)
GH = H // NG       # heads per group
ALPHA = 0.2
BETA = 1.0 - ALPHA
LN_EPS = 1e-5

_CACHE = {}


def build_nc():
    nc = bacc.Bacc("TRN2", target_bir_lowering=False, debug=False)

    x_d = nc.dram_tensor("x", [N, DIN], F32, kind="ExternalInput").ap()
    adj_d = nc.dram_tensor("adj", [N, N], mybir.dt.int32, kind="ExternalInput").ap()
    Wp_d = nc.dram_tensor("Wp", [DIN, F], F32, kind="ExternalInput").ap()
    bp_d = nc.dram_tensor("bp", [F], F32, kind="ExternalInput").ap()
    Wh_d = nc.dram_tensor("W_heads", [L, H, F, F], F32, kind="ExternalInput").ap()
    ah_d = nc.dram_tensor("a_heads", [L, H, 2 * F], F32, kind="ExternalInput").ap()
    Wo_d = nc.dram_tensor("W_out", [L, H * F, F], F32, kind="ExternalInput").ap()
    ao_d = nc.dram_tensor("a_out", [L, 2 * F], F32, kind="ExternalInput").ap()
    g_d = nc.dram_tensor("ln_g", [L, F], F32, kind="ExternalInput").ap()
    b_d = nc.dram_tensor("ln_b", [L, F], F32, kind="ExternalInput").ap()
    out_d = nc.dram_tensor("out", [N, F], F32, kind="ExternalOutput").ap()
    # DRAM bounce buffers for the E1b and 1/den row broadcasts
    ebl_d = [nc.dram_tensor(f"eblk{l}", [16, N], BF16, kind="ExternalOutput").ap()
             for l in range(L)]
    rrd_d = [nc.dram_tensor(f"rrd{l}", [GH, N], BF16, kind="ExternalOutput").ap()
             for l in range(L)]

    with tile.TileContext(nc) as tc, ExitStack() as ctx:
        const = ctx.enter_context(tc.tile_pool(name="const", bufs=1))
        sx = ctx.enter_context(tc.tile_pool(name="sx", bufs=2))
        sproj = ctx.enter_context(tc.tile_pool(name="sproj", bufs=2))
        sbcast = ctx.enter_context(tc.tile_pool(name="sbcast", bufs=9))
        sexp = ctx.enter_context(tc.tile_pool(name="sexp", bufs=7))
        smulti = ctx.enter_context(tc.tile_pool(name="smulti", bufs=9))
        sbig = ctx.enter_context(tc.tile_pool(name="sbig", bufs=3))
        srow = ctx.enter_context(tc.tile_pool(name="srow", bufs=2))
        shd = ctx.enter_context(tc.tile_pool(name="shd", bufs=4))
        smask = ctx.enter_context(tc.tile_pool(name="smask", bufs=4))
        pou = ctx.enter_context(tc.tile_pool(name="pou", bufs=3, space="PSUM"))
        pmisc = ctx.enter_context(tc.tile_pool(name="pmisc", bufs=2, space="PSUM"))
        prow = ctx.enter_context(tc.tile_pool(name="prow", bufs=2, space="PSUM"))

        # ---------------- input DMAs first, spread across all hw queues ----
        x_chunks = []
        for c in range(NCHUNK):
            xc = shd.tile([128, DIN], F32, tag="xchunk")
            nc.sync.dma_start(xc, x_d[bass.ts(c, 128), :])
            x_chunks.append(xc)
        Wh_ball = [const.tile([F, H, F], BF16, name=f"WhB{l}") for l in range(L)]
        Wh0_f = const.tile([F, H, F], F32)
        nc.scalar.dma_start(Wh0_f, Wh_d[0].rearrange("h i o -> i h o"))
        adj_qs = [nc.sync, nc.scalar, nc.sync, nc.scalar]
        adj_raw = []
        for r in range(NCHUNK):
            ai = shd.tile([128, N], mybir.dt.int32, tag="adji", bufs=4)
            adj_qs[r].dma_start(ai, adj_d[bass.ts(r, 128), :])
            adj_raw.append(ai)
        bp_col = const.tile([F, 1], F32)
        nc.sync.dma_start(bp_col, bp_d.rearrange("(f one) -> f one", one=1))

        # ---------------- constants (before gpsimd queue work) ----------------
        ones_row_bf = const.tile([1, N], BF16)
        nc.vector.memset(ones_row_bf, 1.0)
        ones_col_bf = const.tile([128, 1], BF16)
        nc.vector.memset(ones_col_bf, 1.0)
        ident = const.tile([128, 128], F32)
        make_identity(nc, ident)
        ident_bf = const.tile([128, 128], BF16)
        nc.vector.tensor_copy(ident_bf, ident)
        eps_col = const.tile([128, 1], F32)
        nc.vector.memset(eps_col, LN_EPS)
        # onespad[:, j, k] = 1 iff k == j  (den-group lhsT)
        onespad = const.tile([128, GH, GH], BF16)
        nc.vector.memset(onespad, 0.0)
        for j in range(GH):
            nc.vector.memset(onespad[:, j, j:j + 1], 1.0)
        # invFpad[:, c, k] = 1/F iff k == c  (LN 4-partition stats lhsT)
        invFpad = const.tile([128, NCHUNK, NCHUNK], BF16)
        nc.vector.memset(invFpad, 0.0)
        for c in range(NCHUNK):
            nc.vector.memset(invFpad[:, c, c:c + 1], 1.0 / F)
        # sel4[k, j, :] = 1 iff k == j: row-selector lhsT for rank-1
        # broadcasts out of [4, N] tiles (rhs base partition must be 0)
        sel4 = const.tile([4, NCHUNK, 128], BF16)
        nc.gpsimd.memset(sel4, 0.0)
        nc.gpsimd.affine_select(
            out=sel4, in_=sel4, compare_op=OP.not_equal, fill=1.0,
            base=0, pattern=[[-1, NCHUNK], [0, 128]], channel_multiplier=1)
        # sel16[k, h, :] = 1 iff k == 2h: selects the E1b rows of Eblk
        sel16 = const.tile([16, 2, 128], BF16)
        nc.gpsimd.memset(sel16, 0.0)
        nc.gpsimd.affine_select(
            out=sel16, in_=sel16, compare_op=OP.not_equal, fill=1.0,
            base=0, pattern=[[-2, 2], [0, 128]], channel_multiplier=1)

        # gpsimd software-queue weight loads (after the const builds so the
        # identity/selector are ready early for the PE transposes)
        Wp_sb = const.tile([DIN, F], BF16)
        nc.gpsimd.dma_start(Wp_sb, Wp_d)
        ah_ball = const.tile([F, L * H, 2], BF16)
        nc.gpsimd.dma_start(ah_ball, ah_d.rearrange("l h (t f) -> f (l h) t", t=2))
        ah_bf = [[ah_ball[:, l * H + h, :] for h in range(H)] for l in range(L)]
        ao_ball = const.tile([F, L, 2], BF16)
        nc.gpsimd.dma_start(ao_ball, ao_d.rearrange("l (t f) -> f l t", t=2))
        ao_bf = [ao_ball[:, l, :] for l in range(L)]
        gb_all = const.tile([F, 2 * L], F32)
        nc.gpsimd.dma_start(gb_all[:, 0:L], g_d.rearrange("l f -> f l"))
        nc.gpsimd.dma_start(gb_all[:, L:2 * L], b_d.rearrange("l f -> f l"))
        g_col = [gb_all[:, l:l + 1] for l in range(L)]
        b_col = [gb_all[:, L + l:L + l + 1] for l in range(L)]
        Wo_ball = [const.tile([128, H, F], BF16, name=f"WoB{l}") for l in range(L)]
        nc.gpsimd.dma_start(Wo_ball[0], Wo_d[0].rearrange("(c p) f -> p c f", p=128))
        nc.gpsimd.dma_start(Wh_ball[1], Wh_d[1].rearrange("h i o -> i h o"))
        nc.gpsimd.dma_start(Wo_ball[1], Wo_d[1].rearrange("(c p) f -> p c f", p=128))
        Wo_bf = Wo_ball

        Wh_bf = [[Wh_ball[l][:, h, :] for h in range(H)] for l in range(L)]

        # ------------- per-layer weight prep: WhT, Wtilde, WoT, aoWo -------
        WhT_ball = [const.tile([F, H, F], BF16, name=f"WhT{l}") for l in range(L)]
        WoT_ball = [const.tile([F, H, F], BF16, name=f"WoT{l}") for l in range(L)]
        aoWo_ball = const.tile([F, L * H, 2], BF16)
        aoWo = [[aoWo_ball[:, l * H + h, :] for h in range(H)] for l in range(L)]
        Wt_bf = [const.tile([F, 2 * H], BF16, name=f"Wt{l}") for l in range(L)]

        def prep_attn_weights(l):
            if l == 0:
                nc.vector.tensor_copy(Wh_ball[0], Wh0_f)
            for h in range(H):
                pt = pou.tile([128, 128], BF16, tag="oU")
                nc.tensor.transpose(pt, Wh_bf[l][h], ident_bf)
                # layer-0 prep runs pre-dense: keep ACT free for the
                # Eblk/e1b/C_e2 chain that gates the first score op
                if l > 0 and h % 2 == 0:
                    nc.scalar.activation(WhT_ball[l][:, h, :], pt, AF.Copy)
                else:
                    nc.vector.tensor_copy(WhT_ball[l][:, h, :], pt)
            pw = prow.tile([128, 2 * H], F32, tag="prow")
            for h in range(H):
                nc.tensor.matmul(pw[:, 2 * h:2 * h + 2], WhT_ball[l][:, h, :],
                                 ah_bf[l][h], start=True, stop=True)
            nc.scalar.activation(Wt_bf[l], pw, AF.Copy)

        def prep_out_weights(l):
            for h in range(H):
                pt = pou.tile([128, 128], BF16, tag="oU")
                nc.tensor.transpose(pt, Wo_ball[l][:, h, :], ident_bf)
                if h % 2 == 0:
                    nc.scalar.activation(WoT_ball[l][:, h, :], pt, AF.Copy)
                else:
                    nc.vector.tensor_copy(WoT_ball[l][:, h, :], pt)
            paw = prow.tile([128, 2 * H], F32, tag="prow")
            for h in range(H):
                nc.tensor.matmul(paw[:, 2 * h:2 * h + 2],
                                 WoT_ball[l][:, h, :], ao_bf[l],
                                 start=True, stop=True)
            nc.scalar.activation(
                aoWo_ball[:, l * H:(l + 1) * H, :].rearrange("p h t -> p (h t)"),
                paw, AF.Copy)

        # ---------------- x -> xT, input projection ----------------
        xT = const.tile([DIN, N], BF16)
        ph = pmisc.tile([128, N], F32, tag="pbig")
        hT_bf = sbig.tile([128, N], BF16, tag="hTb", bufs=2)
        for c in range(NCHUNK):
            xb = shd.tile([128, DIN], BF16, tag="xchb")
            nc.vector.tensor_copy(xb, x_chunks[c])
            pt = pmisc.tile([DIN, 128], BF16, tag="pbig")
            nc.tensor.transpose(pt, xb, ident_bf)
            nc.scalar.activation(xT[:, bass.ts(c, 128)], pt, AF.Copy)
            nc.tensor.matmul(ph[:, bass.ts(c, 128)], Wp_sb, xT[:, bass.ts(c, 128)],
                             start=True, stop=True)
            nc.scalar.activation(hT_bf[:, bass.ts(c, 128)], ph[:, bass.ts(c, 128)],
                                 AF.Relu, bias=bp_col)
        hT = hT_bf

        prep_attn_weights(0)

        # ---------------- adj -> maskT (bf16, transposed) ----------------
        adj_f = []
        for r in range(NCHUNK):
            af = smask.tile([128, N], BF16, tag="adjf")
            nc.vector.tensor_copy(af, adj_raw[r])
            adj_f.append(af)
        maskT_all = const.tile([128, NCHUNK, N], BF16)
        maskT = [maskT_all[:, c, :] for c in range(NCHUNK)]
        for r in range(NCHUNK):
            for c in range(NCHUNK):
                pm = pmisc.tile([128, 128], BF16, tag="pbig")
                nc.tensor.transpose(pm, adj_f[r][:, bass.ts(c, 128)], ident_bf)
                nc.vector.tensor_copy(maskT[c][:, bass.ts(r, 128)], pm)

        # ---------------- layers ----------------
        for l in range(L):
            residT = hT
            # --- rows for all heads: s12[2h] = s1_h, s12[2h+1] = s2_h
            s12_ps = prow.tile([2 * H, N], F32, tag="prow")
            nc.tensor.matmul(s12_ps, Wt_bf[l], hT_bf, start=True, stop=True)
            Eblk = sx.tile([16, N], BF16, tag="Eblk")   # exp(+beta*s): rows 2h = E1b
            nc.scalar.activation(Eblk, s12_ps, AF.Exp, scale=BETA)
            # E1b broadcasts: one DRAM bounce write of all rows, then one
            # stride-0 broadcast read per head, spread across DMA queues
            dmaq = [nc.sync, nc.scalar, nc.gpsimd]
            nc.sync.dma_start(ebl_d[l], Eblk)
            e1b = []
            for h in range(H):
                eb = sbcast.tile([128, N], BF16, tag="e1b", bufs=9)
                if h < 2:
                    # first heads via PE rank-1 + ACT evac: available ~1.5us
                    # earlier than the DRAM bounce, starts the dense phase
                    ebp = pmisc.tile([128, N], F32, tag="pbig")
                    nc.tensor.matmul(ebp, sel16[:, h, :], Eblk,
                                     start=True, stop=True)
                    nc.scalar.activation(eb, ebp, AF.Copy)
                else:
                    row = ebl_d[l][2 * h, :]
                    src_bc = bass.AP(tensor=row.tensor, offset=row.offset,
                                     ap=[[0, 128], [1, N]])
                    dmaq[h % 3].dma_start(eb, src_bc)
                e1b.append(eb)
            # --- s2 columns directly via tiny matmuls (no transposes)
            Wt2 = Wt_bf[l].rearrange("i (h t) -> i t h", t=2)[:, 1, :]
            cps = prow.tile([128, NCHUNK, 8], F32, tag="prow")
            for c in range(NCHUNK):
                nc.tensor.matmul(cps[:, c, :], hT_bf[:, bass.ts(c, 128)], Wt2,
                                 start=True, stop=True)
            C_e2f = sx.tile([128, NCHUNK, 8], F32, tag="Ce2f")
            nc.scalar.activation(C_e2f, cps, AF.Exp, scale=1.0)
            C_e2a = sx.tile([128, NCHUNK, 8], F32, tag="Ce2a")
            nc.scalar.activation(C_e2a, cps, AF.Exp, scale=ALPHA)

            def e2_col(h, c):
                return C_e2f[:, c, h:h + 1]

            def e2a_col(h, c):
                return C_e2a[:, c, h:h + 1]

            # --- projN: batched over heads (2 x 512-free MMs per chunk),
            # plain batched evacuation (E2 lives in s_t now)
            projAll = sproj.tile([128, NCHUNK, H * 128], BF16, tag="projAll",
                                 name=f"pa{l}", bufs=1)
            WhV = Wh_ball[l].rearrange("i h f -> i (h f)")
            for c in range(NCHUNK):
                for g in range(2):
                    pN = pmisc.tile([128, N], F32, tag="pbig")
                    nc.tensor.matmul(pN, hT_bf[:, bass.ts(c, 128)],
                                     WhV[:, bass.ts(g, 512)], start=True, stop=True)
                    dst = projAll[:, c, g * 512:(g + 1) * 512]
                    nc.scalar.activation(dst, pN, AF.Copy)

            def proj_ct(h, c):
                return projAll[:, c, h * 128:(h + 1) * 128]

            if l == 0:
                # deferred weight prep runs inside layer-0's dense phase
                prep_out_weights(0)
                prep_attn_weights(1)
                prep_out_weights(1)

            # --- attention per group of GH heads
            pous = [None] * H
            reps = [None] * H
            for g in range(NG):
                deng_ps = prow.tile([GH, N], F32, tag="deng", bufs=1)
                for j in range(GH):
                    h = g * GH + j
                    tten = sexp.tile([128, NCHUNK, N], BF16, tag="tten", bufs=3)
                    for c in range(NCHUNK):
                        nc.vector.tensor_scalar(tten[:, c, :], e1b[h],
                                                e2_col(h, c), e2a_col(h, c),
                                                OP.mult, OP.max)
                    s_t = sexp.tile([128, NCHUNK, N], BF16, tag="s_t")
                    nc.vector.tensor_tensor(s_t, tten, maskT_all, OP.mult)
                    for c in range(NCHUNK):
                        nc.tensor.matmul(deng_ps, onespad[:, j, :], s_t[:, c, :],
                                         start=(j == 0 and c == 0),
                                         stop=(j == GH - 1 and c == NCHUNK - 1))
                    pou_ps = pou.tile([128, N], F32, tag="oU")
                    for c in range(NCHUNK):
                        nc.tensor.matmul(pou_ps, proj_ct(h, c), s_t[:, c, :],
                                         start=(c == 0), stop=(c == NCHUNK - 1))
                    pob = smulti.tile([128, N], BF16, tag="pob", bufs=8)
                    nc.scalar.activation(pob, pou_ps, AF.Copy)
                    pous[h] = pob
                rr4 = srow.tile([GH, N], F32, tag="rr4")
                nc.vector.reciprocal_approx_fast(rr4, deng_ps)
                rr4_bf = srow.tile([GH, N], BF16, tag="rr4b")
                nc.vector.tensor_copy(rr4_bf, rr4)
                for j in range(GH):
                    h = g * GH + j
                    rep_ps = pmisc.tile([128, N], F32, tag="pbig")
                    nc.tensor.matmul(rep_ps, sel4[:, j, :], rr4_bf,
                                     start=True, stop=True)
                    rp = sbcast.tile([128, N], BF16, tag="rep", bufs=6)
                    nc.scalar.activation(rp, rep_ps, AF.Copy)
                    reps[h] = rp

            # --- normalize + ELU per head
            multiT = []
            for h in range(H):
                outT = sbig.tile([128, N], BF16, tag="outT", bufs=3)
                nc.vector.tensor_tensor(outT, pous[h], reps[h], OP.mult)
                ex = shd.tile([128, N], BF16, tag="elu_ex")
                nc.scalar.activation(ex, outT, AF.Exp)
                ex2 = shd.tile([128, N], BF16, tag="elu_ex2", bufs=3)
                nc.vector.tensor_scalar(ex2, ex, 1.0, -1.0, OP.min, OP.add)
                mh = smulti.tile([128, N], BF16, tag="multi")
                nc.vector.tensor_tensor(mh, outT, ex2, OP.max)
                multiT.append(mh)

            # --- W_out projection (h2 in F-layout)
            ph2 = pou.tile([128, N], F32, tag="oU")
            for h in range(H):
                nc.tensor.matmul(ph2, Wo_bf[l][:, h, :], multiT[h],
                                 start=(h == 0), stop=(h == H - 1))
            h2_bf = sbig.tile([128, N], BF16, tag="h2b", bufs=2)
            nc.scalar.activation(h2_bf, ph2, AF.Copy)

            # --- single out-attention: s12o accumulated per head from multiT
            # via aoWo (no wait on the full h2_bf)
            s12o_ps = prow.tile([2, N], F32, tag="prow")
            for h in range(H):
                nc.tensor.matmul(s12o_ps, aoWo[l][h], multiT[h],
                                 start=(h == 0), stop=(h == H - 1))
            Xo_b = sx.tile([1, N], BF16, tag="Xo_b")    # E1b_o row
            nc.scalar.activation(Xo_b, s12o_ps[0:1, :], AF.Exp, scale=BETA)
            ebo_ps = pmisc.tile([128, N], F32, tag="pbig")
            nc.tensor.matmul(ebo_ps, ones_row_bf[:, 0:128], Xo_b,
                             start=True, stop=True)
            e1bo = sbcast.tile([128, N], BF16, tag="e1b", bufs=9)
            nc.scalar.activation(e1bo, ebo_ps, AF.Copy)
            so_ps = prow.tile([128, NCHUNK, 2], F32, tag="prow")
            for c in range(NCHUNK):
                nc.tensor.matmul(so_ps[:, c, :], h2_bf[:, bass.ts(c, 128)],
                                 ao_bf[l], start=True, stop=True)
            Co_e2f = sx.tile([128, NCHUNK, 2], F32, tag="Coe2f")
            nc.scalar.activation(Co_e2f, so_ps, AF.Exp, scale=1.0)
            Co_e2a = sx.tile([128, NCHUNK, 2], F32, tag="Coe2a")
            nc.scalar.activation(Co_e2a, so_ps, AF.Exp, scale=ALPHA)
            # h2N via PE transposes of h2_bf, plain evac (E2_o lives in s_to)
            h2Np = sproj.tile([128, NCHUNK, 128], BF16, tag="h2Np")
            for c in range(NCHUNK):
                pm = pmisc.tile([128, 128], BF16, tag="pbig")
                nc.tensor.transpose(pm, h2_bf[:, bass.ts(c, 128)], ident_bf)
                nc.scalar.activation(h2Np[:, c, :], pm, AF.Copy)
            # out-att scores
            tto = sexp.tile([128, NCHUNK, N], BF16, tag="tten", bufs=3)
            for c in range(NCHUNK):
                nc.vector.tensor_scalar(tto[:, c, :], e1bo,
                                        Co_e2f[:, c, 1:2], Co_e2a[:, c, 1:2],
                                        OP.mult, OP.max)
            s_to = sexp.tile([128, NCHUNK, N], BF16, tag="s_t")
            nc.vector.tensor_tensor(s_to[:, 0:2, :], tto[:, 0:2, :],
                                    maskT_all[:, 0:2, :], OP.mult)
            nc.vector.tensor_tensor(s_to[:, 2:4, :], tto[:, 2:4, :],
                                    maskT_all[:, 2:4, :], OP.mult)
            deno_ps = prow.tile([1, N], F32, tag="prow")
            for c in range(NCHUNK):
                nc.tensor.matmul(deno_ps, ones_col_bf, s_to[:, c, :],
                                 start=(c == 0), stop=(c == NCHUNK - 1))
            pouo_ps = pou.tile([128, N], F32, tag="oU")
            for c in range(NCHUNK):
                nc.tensor.matmul(pouo_ps, h2Np[:, c, :], s_to[:, c, :],
                                 start=(c == 0), stop=(c == NCHUNK - 1))
            rro = srow.tile([1, N], F32, tag="rro")
            nc.vector.reciprocal_approx_fast(rro, deno_ps)
            rro_bf = srow.tile([1, N], BF16, tag="rrob")
            nc.vector.tensor_copy(rro_bf, rro)
            rpo_ps = pmisc.tile([128, N], F32, tag="pbig")
            nc.tensor.matmul(rpo_ps, ones_row_bf[:, 0:128], rro_bf,
                             start=True, stop=True)
            pobo = smulti.tile([128, N], BF16, tag="pobo", bufs=2)
            nc.scalar.activation(pobo, pouo_ps, AF.Copy)
            outsT = sbig.tile([128, N], BF16, tag="outsT", bufs=2)
            nc.vector.tensor_tensor(outsT, pobo, rpo_ps, OP.mult)

            # ---- residual + LN over partition dim (bf16 stream) ----
            xs = sbig.tile([128, N], BF16, tag="xs", bufs=2)
            nc.vector.tensor_tensor(xs, outsT, residT, OP.add)
            xsq = sbig.tile([128, N], BF16, tag="xsq", bufs=2)
            nc.scalar.activation(xsq, xs, AF.Square)
            # 4-partition stats: row c of [4,128] = mean/meansq of chunk c
            pmu4 = prow.tile([NCHUNK, 128], F32, tag="prow")
            for c in range(NCHUNK):
                nc.tensor.matmul(pmu4, invFpad[:, c, :], xs[:, bass.ts(c, 128)],
                                 start=(c == 0), stop=(c == NCHUNK - 1))
            psq4 = prow.tile([NCHUNK, 128], F32, tag="prow")
            for c in range(NCHUNK):
                nc.tensor.matmul(psq4, invFpad[:, c, :], xsq[:, bass.ts(c, 128)],
                                 start=(c == 0), stop=(c == NCHUNK - 1))
            mu2 = srow.tile([NCHUNK, 128], F32, tag="rowL", bufs=4)
            nc.scalar.activation(mu2, pmu4, AF.Square)
            nmu4_bf = srow.tile([NCHUNK, 128], BF16, tag="rowLb", bufs=2)
            nc.vector.tensor_scalar_mul(nmu4_bf, pmu4, -1.0)
            # rstd = (var+eps)^-1/2 via int32-view seed + 2 Newton steps
            # (avoids Ln/Exp -> no ACT table switching)
            ve = srow.tile([NCHUNK, 128], F32, tag="rowL", bufs=4)
            nc.vector.scalar_tensor_tensor(ve, psq4, LN_EPS, mu2,
                                           OP.add, OP.subtract)
            y0 = srow.tile([NCHUNK, 128], F32, tag="rowL", bufs=4)
            nc.vector.tensor_scalar(y0.bitcast(mybir.dt.int32),
                                    ve.bitcast(mybir.dt.int32),
                                    -0.5, 1597463007.0, OP.mult, OP.add)
            w1 = srow.tile([NCHUNK, 128], F32, tag="rowL", bufs=4)
            nc.vector.tensor_tensor(w1, y0, y0, OP.mult)
            z1 = srow.tile([NCHUNK, 128], F32, tag="rowL", bufs=4)
            nc.vector.scalar_tensor_tensor(z1, ve, -0.5, w1, OP.mult, OP.mult)
            y1 = srow.tile([NCHUNK, 128], F32, tag="rowL", bufs=4)
            nc.vector.scalar_tensor_tensor(y1, z1, 1.5, y0, OP.add, OP.mult)
            w2 = srow.tile([NCHUNK, 128], F32, tag="rowL", bufs=4)
            nc.vector.tensor_tensor(w2, y1, y1, OP.mult)
            z2 = srow.tile([NCHUNK, 128], F32, tag="rowL", bufs=4)
            nc.vector.scalar_tensor_tensor(z2, ve, -0.5, w2, OP.mult, OP.mult)
            rstd4_bf = srow.tile([NCHUNK, 128], BF16, tag="rowLb", bufs=2)
            nc.vector.scalar_tensor_tensor(rstd4_bf, z2, 1.5, y1,
                                           OP.add, OP.mult)
            # (2 Newton steps: rstd to ~1e-5 rel; plenty for the 2e-2 gate)
            rep_rstd = pmisc.tile([128, N], F32, tag="pbig")
            rep_nmu = pmisc.tile([128, N], F32, tag="pbig")
            for c in range(NCHUNK):
                nc.tensor.matmul(rep_rstd[:, bass.ts(c, 128)],
                                 sel4[:, c, :], rstd4_bf,
                                 start=True, stop=True)
                nc.tensor.matmul(rep_nmu[:, bass.ts(c, 128)],
                                 sel4[:, c, :], nmu4_bf,
                                 start=True, stop=True)
            rep_nmu_b = sbcast.tile([128, N], BF16, tag="repl", bufs=2)
            nc.scalar.activation(rep_nmu_b, rep_nmu, AF.Copy)
            rep_rstd_b = sbcast.tile([128, N], BF16, tag="repl", bufs=2)
            nc.scalar.activation(rep_rstd_b, rep_rstd, AF.Copy)
            u = sbig.tile([128, N], BF16, tag="u", bufs=2)
            nc.vector.tensor_tensor(u, xs, rep_nmu_b, OP.add)
            t2 = sbig.tile([128, N], BF16, tag="t2", bufs=2)
            nc.vector.tensor_tensor(t2, u, rep_rstd_b, OP.mult)
            hT_bf = sbig.tile([128, N], BF16, tag="hTb", bufs=2)
            fn = AF.Relu if l < L - 1 else AF.Identity
            nc.scalar.activation(hT_bf, t2, fn, scale=g_col[l], bias=b_col[l])
            hT = hT_bf

        # ---------------- output: transpose back ----------------
        for c in range(NCHUNK):
            po = pmisc.tile([128, 128], BF16, tag="pbig")
            nc.tensor.transpose(po, hT[:, bass.ts(c, 128)], ident_bf)
            osb = shd.tile([128, 128], F32, tag="osb")
            nc.scalar.activation(osb, po, AF.Copy)
            nc.sync.dma_start(out_d[bass.ts(c, 128), :], osb)

    nc.compile()
    return nc


def _get_nc():
    if "nc" not in _CACHE:
        _CACHE["nc"] = build_nc()
    return _CACHE["nc"]


def kernel(**inputs) -> np.ndarray:
    nc = _get_nc()
    shared = {k: np.ascontiguousarray(np.asarray(inputs[k], dtype=np.float32))
              for k in ("Wp", "bp", "W_heads", "a_heads", "W_out", "a_out",
                        "ln_g", "ln_b")}
    x = np.asarray(inputs["x"], dtype=np.float32)
    adj = np.asarray(inputs["adj"], dtype=np.int32)
    in_maps = [dict(x=np.ascontiguousarray(x[b]),
                    adj=np.ascontiguousarray(adj[b]), **shared)
               for b in range(B)]
    res = run_bass_kernel_spmd(nc, in_maps, core_ids=list(range(B)))
    return np.stack([res.results[b]["out"] for b in range(B)])


if __name__ == "__main__":
    rng = np.random.default_rng(0)
    inputs = dict(
        x=rng.normal(size=(B, N, DIN)).astype(np.float32),
        adj=rng.integers(0, 2, size=(B, N, N)).astype(np.int32),
        Wp=(rng.normal(size=(DIN, F)) * 0.12).astype(np.float32),
        bp=np.zeros(F, dtype=np.float32),
        W_heads=(rng.normal(size=(L, H, F, F)) * 0.08).astype(np.float32),
        a_heads=(rng.normal(size=(L, H, 2 * F)) * 0.08).astype(np.float32),
        W_out=(rng.normal(size=(L, H * F, F)) * 0.03).astype(np.float32),
        a_out=(rng.normal(size=(L, 2 * F)) * 0.08).astype(np.float32),
        ln_g=np.ones((L, F), dtype=np.float32),
        ln_b=np.zeros((L, F), dtype=np.float32),
    )
    out = kernel(**inputs)
    print("out", out.shape, out.dtype, np.abs(out).max())


# revision 30
# speedup vs baseline: 1.0271x; 1.0064x over previous
"""GAT spatio-temporal model Trainium2 kernel (v6).

Sharding: data-parallel over batch B=8 -> 8 NeuronCores (1 graph each).

Attention factorization (exact): with E = exp(s), Ea = exp(alpha*s),
exp(lrelu(s1[n]+s2[m])) = max(E1[n]E2[m], E1a[n]E2a[m]).  Dividing by
E1a[n] (constant along the softmax axis, cancels):
    p[m,n] = max(E2[m]*E1b[n], E2a[m]) * mask[n,m],  E1b = exp(beta*s1)
so E2 is folded INTO the score tensor (v6):
 - t = tensor_scalar(e1b_bcast, *E2[m], max E2a[m]) -- 2x DVE mode
 - s_t = t * maskT (one batched [128,4N] tensor_tensor per head)
 - num = sum_m projN_plain[m,F] s_t[m,n]  (plain batched PSUM evacs)
 - den = sum_m s_t[m,n] via zero-padded ones lhsT, 4 heads -> one [4,N]
   PSUM tile -> ONE reciprocal + cast per group.
 - 1/den and E1b_o broadcasts via PE rank-1 + ACT evac (low latency).
 - h2N via PE transposes of h2_bf (not 32 small matmuls).
 - LN: stats for the 4 n-chunks land on 4 PSUM partitions (zero-padded
   1/F lhsT) so row ops run 4x fewer elements; affine+ReLU fused into
   one ACT (scale=g, bias=b per partition).

Shapes (hardcoded): B=8, N=512, Din=64, H=8, F=128, L=2.
"""
import os
import numpy as np
from contextlib import ExitStack

import concourse.bass as bass
import concourse.tile as tile
from concourse import bacc, mybir
from concourse.bass_utils import run_bass_kernel_spmd
from concourse.masks import make_identity

F32 = mybir.dt.float32
BF16 = mybir.dt.bfloat16
AF = mybir.ActivationFunctionType
OP = mybir.AluOpType

B, N, DIN, H, F, L = 8, 512, 64, 8, 128, 2
NCHUNK = N // 128  # 4
NG = 2             # den groups per layer (4 heads each)
GH = H // NG       # heads per group
ALPHA = 0.2
BETA = 1.0 - ALPHA
LN_EPS = 1e-5

_CACHE = {}


def build_nc():
    nc = bacc.Bacc("TRN2", target_bir_lowering=False, debug=False)

    x_d = nc.dram_tensor("x", [N, DIN], F32, kind="ExternalInput").ap()
    adj_d = nc.dram_tensor("adj", [N, N], mybir.dt.int32, kind="ExternalInput").ap()
    Wp_d = nc.dram_tensor("Wp", [DIN, F], F32, kind="ExternalInput").ap()
    bp_d = nc.dram_tensor("bp", [F], F32, kind="ExternalInput").ap()
    Wh_d = nc.dram_tensor("W_heads", [L, H, F, F], F32, kind="ExternalInput").ap()
    ah_d = nc.dram_tensor("a_heads", [L, H, 2 * F], F32, kind="ExternalInput").ap()
    Wo_d = nc.dram_tensor("W_out", [L, H * F, F], F32, kind="ExternalInput").ap()
    ao_d = nc.dram_tensor("a_out", [L, 2 * F], F32, kind="ExternalInput").ap()
    g_d = nc.dram_tensor("ln_g", [L, F], F32, kind="ExternalInput").ap()
    b_d = nc.dram_tensor("ln_b", [L, F], F32, kind="ExternalInput").ap()
    out_d = nc.dram_tensor("out", [N, F], F32, kind="ExternalOutput").ap()
    # DRAM bounce buffers for the E1b and 1/den row broadcasts
    ebl_d = [nc.dram_tensor(f"eblk{l}", [16, N], BF16, kind="ExternalOutput").ap()
             for l in range(L)]
    rrd_d = [nc.dram_tensor(f"rrd{l}", [GH, N], BF16, kind="ExternalOutput").ap()
             for l in range(L)]

    with tile.TileContext(nc) as tc, ExitStack() as ctx:
        const = ctx.enter_context(tc.tile_pool(name="const", bufs=1))
        sx = ctx.enter_context(tc.tile_pool(name="sx", bufs=2))
        sproj = ctx.enter_context(tc.tile_pool(name="sproj", bufs=2))
        sbcast = ctx.enter_context(tc.tile_pool(name="sbcast", bufs=9))
        sexp = ctx.enter_context(tc.tile_pool(name="sexp", bufs=7))
        smulti = ctx.enter_context(tc.tile_pool(name="smulti", bufs=9))
        sbig = ctx.enter_context(tc.tile_pool(name="sbig", bufs=3))
        srow = ctx.enter_context(tc.tile_pool(name="srow", bufs=2))
        shd = ctx.enter_context(tc.tile_pool(name="shd", bufs=4))
        smask = ctx.enter_context(tc.tile_pool(name="smask", bufs=4))
        pou = ctx.enter_context(tc.tile_pool(name="pou", bufs=3, space="PSUM"))
        pmisc = ctx.enter_context(tc.tile_pool(name="pmisc", bufs=2, space="PSUM"))
        prow = ctx.enter_context(tc.tile_pool(name="prow", bufs=2, space="PSUM"))

        # ---------------- input DMAs first, spread across all hw queues ----
        x_chunks = []
        for c in range(NCHUNK):
            xc = shd.tile([128, DIN], F32, tag="xchunk")
            nc.sync.dma_start(xc, x_d[bass.ts(c, 128), :])
            x_chunks.append(xc)
        Wh_ball = [const.tile([F, H, F], BF16, name=f"WhB{l}") for l in range(L)]
        Wh0_f = const.tile([F, H, F], F32)
        nc.scalar.dma_start(Wh0_f, Wh_d[0].rearrange("h i o -> i h o"))
        adj_qs = [nc.sync, nc.scalar, nc.sync, nc.scalar]
        adj_raw = []
        for r in range(NCHUNK):
            ai = shd.tile([128, N], mybir.dt.int32, tag="adji", bufs=4)
            adj_qs[r].dma_start(ai, adj_d[bass.ts(r, 128), :])
            adj_raw.append(ai)
        bp_col = const.tile([F, 1], F32)
        nc.sync.dma_start(bp_col, bp_d.rearrange("(f one) -> f one", one=1))

        # ---------------- constants (before gpsimd queue work) ----------------
        ones_row_bf = const.tile([1, N], BF16)
        nc.vector.memset(ones_row_bf, 1.0)
        ones_col_bf = const.tile([128, 1], BF16)
        nc.vector.memset(ones_col_bf, 1.0)
        ident = const.tile([128, 128], F32)
        make_identity(nc, ident)
        ident_bf = const.tile([128, 128], BF16)
        nc.vector.tensor_copy(ident_bf, ident)
        eps_col = const.tile([128, 1], F32)
        nc.vector.memset(eps_col, LN_EPS)
        # onespad[:, j, k] = 1 iff k == j  (den-group lhsT)
        onespad = const.tile([128, GH, GH], BF16)
        nc.vector.memset(onespad, 0.0)
        for j in range(GH):
            nc.vector.memset(onespad[:, j, j:j + 1], 1.0)
        # invFpad[:, c, k] = 1/F iff k == c  (LN 4-partition stats lhsT)
        invFpad = const.tile([128, NCHUNK, NCHUNK], BF16)
        nc.vector.memset(invFpad, 0.0)
        for c in range(NCHUNK):
            nc.vector.memset(invFpad[:, c, c:c + 1], 1.0 / F)
        # sel4[k, j, :] = 1 iff k == j: row-selector lhsT for rank-1
        # broadcasts out of [4, N] tiles (rhs base partition must be 0)
        sel4 = const.tile([4, NCHUNK, 128], BF16)
        nc.gpsimd.memset(sel4, 0.0)
        nc.gpsimd.affine_select(
            out=sel4, in_=sel4, compare_op=OP.not_equal, fill=1.0,
            base=0, pattern=[[-1, NCHUNK], [0, 128]], channel_multiplier=1)
        # sel16[k, h, :] = 1 iff k == 2h: selects the E1b rows of Eblk
        sel16 = const.tile([16, 2, 128], BF16)
        nc.gpsimd.memset(sel16, 0.0)
        nc.gpsimd.affine_select(
            out=sel16, in_=sel16, compare_op=OP.not_equal, fill=1.0,
            base=0, pattern=[[-2, 2], [0, 128]], channel_multiplier=1)

        # gpsimd software-queue weight loads (after the const builds so the
        # identity/selector are ready early for the PE transposes)
        Wp_sb = const.tile([DIN, F], BF16)
        nc.gpsimd.dma_start(Wp_sb, Wp_d)
        ah_ball = const.tile([F, L * H, 2], BF16)
        nc.gpsimd.dma_start(ah_ball, ah_d.rearrange("l h (t f) -> f (l h) t", t=2))
        ah_bf = [[ah_ball[:, l * H + h, :] for h in range(H)] for l in range(L)]
        ao_ball = const.tile([F, L, 2], BF16)
        nc.gpsimd.dma_start(ao_ball, ao_d.rearrange("l (t f) -> f l t", t=2))
        ao_bf = [ao_ball[:, l, :] for l in range(L)]
        gb_all = const.tile([F, 2 * L], F32)
        nc.gpsimd.dma_start(gb_all[:, 0:L], g_d.rearrange("l f -> f l"))
        nc.gpsimd.dma_start(gb_all[:, L:2 * L], b_d.rearrange("l f -> f l"))
        g_col = [gb_all[:, l:l + 1] for l in range(L)]
        b_col = [gb_all[:, L + l:L + l + 1] for l in range(L)]
        Wo_ball = [const.tile([128, H, F], BF16, name=f"WoB{l}") for l in range(L)]
        nc.gpsimd.dma_start(Wo_ball[0], Wo_d[0].rearrange("(c p) f -> p c f", p=128))
        nc.gpsimd.dma_start(Wh_ball[1], Wh_d[1].rearrange("h i o -> i h o"))
        nc.gpsimd.dma_start(Wo_ball[1], Wo_d[1].rearrange("(c p) f -> p c f", p=128))
        Wo_bf = Wo_ball

        Wh_bf = [[Wh_ball[l][:, h, :] for h in range(H)] for l in range(L)]

        # ------------- per-layer weight prep: WhT, Wtilde, WoT, aoWo -------
        WhT_ball = [const.tile([F, H, F], BF16, name=f"WhT{l}") for l in range(L)]
        WoT_ball = [const.tile([F, H, F], BF16, name=f"WoT{l}") for l in range(L)]
        aoWo_ball = const.tile([F, L * H, 2], BF16)
        aoWo = [[aoWo_ball[:, l * H + h, :] for h in range(H)] for l in range(L)]
        Wt_bf = [const.tile([F, 2 * H], BF16, name=f"Wt{l}") for l in range(L)]

        def prep_attn_weights(l):
            if l == 0:
                nc.vector.tensor_copy(Wh_ball[0], Wh0_f)
            for h in range(H):
                pt = pou.tile([128, 128], BF16, tag="oU")
                nc.tensor.transpose(pt, Wh_bf[l][h], ident_bf)
                # layer-0 prep runs pre-dense: keep ACT free for the
                # Eblk/e1b/C_e2 chain that gates the first score op
                if l > 0 and h % 2 == 0:
                    nc.scalar.activation(WhT_ball[l][:, h, :], pt, AF.Copy)
                else:
                    nc.vector.tensor_copy(WhT_ball[l][:, h, :], pt)
            pw = prow.tile([128, 2 * H], F32, tag="prow")
            for h in range(H):
                nc.tensor.matmul(pw[:, 2 * h:2 * h + 2], WhT_ball[l][:, h, :],
                                 ah_bf[l][h], start=True, stop=True)
            nc.scalar.activation(Wt_bf[l], pw, AF.Copy)

        def prep_out_weights(l):
            for h in range(H):
                pt = pou.tile([128, 128], BF16, tag="oU")
                nc.tensor.transpose(pt, Wo_ball[l][:, h, :], ident_bf)
                if h % 2 == 0:
                    nc.scalar.activation(WoT_ball[l][:, h, :], pt, AF.Copy)
                else:
                    nc.vector.tensor_copy(WoT_ball[l][:, h, :], pt)
            paw = prow.tile([128, 2 * H], F32, tag="prow")
            for h in range(H):
                nc.tensor.matmul(paw[:, 2 * h:2 * h + 2],
                                 WoT_ball[l][:, h, :], ao_bf[l],
                                 start=True, stop=True)
            nc.scalar.activation(
                aoWo_ball[:, l * H:(l + 1) * H, :].rearrange("p h t -> p (h t)"),
                paw, AF.Copy)

        # ---------------- x -> xT, input projection ----------------
        xT = const.tile([DIN, N], BF16)
        ph = pmisc.tile([128, N], F32, tag="pbig")
        hT_bf = sbig.tile([128, N], BF16, tag="hTb", bufs=2)
        for c in range(NCHUNK):
            xb = shd.tile([128, DIN], BF16, tag="xchb")
            nc.vector.tensor_copy(xb, x_chunks[c])
            pt = pmisc.tile([DIN, 128], BF16, tag="pbig")
            nc.tensor.transpose(pt, xb, ident_bf)
            nc.scalar.activation(xT[:, bass.ts(c, 128)], pt, AF.Copy)
            nc.tensor.matmul(ph[:, bass.ts(c, 128)], Wp_sb, xT[:, bass.ts(c, 128)],
                             start=True, stop=True)
            nc.scalar.activation(hT_bf[:, bass.ts(c, 128)], ph[:, bass.ts(c, 128)],
                                 AF.Relu, bias=bp_col)
        hT = hT_bf

        prep_attn_weights(0)

        # ---------------- adj -> maskT (bf16, transposed) ----------------
        adj_f = []
        for r in range(NCHUNK):
            af = smask.tile([128, N], BF16, tag="adjf")
            nc.vector.tensor_copy(af, adj_raw[r])
            adj_f.append(af)
        maskT_all = const.tile([128, NCHUNK, N], BF16)
        maskT = [maskT_all[:, c, :] for c in range(NCHUNK)]
        for r in range(NCHUNK):
            for c in range(NCHUNK):
                pm = pmisc.tile([128, 128], BF16, tag="pbig")
                nc.tensor.transpose(pm, adj_f[r][:, bass.ts(c, 128)], ident_bf)
                nc.vector.tensor_copy(maskT[c][:, bass.ts(r, 128)], pm)

        # ---------------- layers ----------------
        for l in range(L):
            residT = hT
            # --- rows for all heads: s12[2h] = s1_h, s12[2h+1] = s2_h
            s12_ps = prow.tile([2 * H, N], F32, tag="prow")
            nc.tensor.matmul(s12_ps, Wt_bf[l], hT_bf, start=True, stop=True)
            Eblk = sx.tile([16, N], BF16, tag="Eblk")   # exp(+beta*s): rows 2h = E1b
            nc.scalar.activation(Eblk, s12_ps, AF.Exp, scale=BETA)
            # E1b broadcasts: one DRAM bounce write of all rows, then one
            # stride-0 broadcast read per head, spread across DMA queues
            dmaq = [nc.sync, nc.gpsimd]
            nc.sync.dma_start(ebl_d[l], Eblk)
            e1b = []
            for h in range(H):
                eb = sbcast.tile([128, N], BF16, tag="e1b", bufs=9)
                if h < 2:
                    # first heads via PE rank-1 + ACT evac: available ~1.5us
                    # earlier than the DRAM bounce, starts the dense phase
                    ebp = pmisc.tile([128, N], F32, tag="pbig")
                    nc.tensor.matmul(ebp, sel16[:, h, :], Eblk,
                                     start=True, stop=True)
                    nc.scalar.activation(eb, ebp, AF.Copy)
                else:
                    row = ebl_d[l][2 * h, :]
                    src_bc = bass.AP(tensor=row.tensor, offset=row.offset,
                                     ap=[[0, 128], [1, N]])
                    dmaq[h % 2].dma_start(eb, src_bc)
                e1b.append(eb)
            # --- s2 columns directly via tiny matmuls (no transposes)
            Wt2 = Wt_bf[l].rearrange("i (h t) -> i t h", t=2)[:, 1, :]
            cps = prow.tile([128, NCHUNK, 8], F32, tag="prow")
            for c in range(NCHUNK):
                nc.tensor.matmul(cps[:, c, :], hT_bf[:, bass.ts(c, 128)], Wt2,
                                 start=True, stop=True)
            C_e2f = sx.tile([128, NCHUNK, 8], F32, tag="Ce2f")
            nc.scalar.activation(C_e2f, cps, AF.Exp, scale=1.0)
            C_e2a = sx.tile([128, NCHUNK, 8], F32, tag="Ce2a")
            nc.scalar.activation(C_e2a, cps, AF.Exp, scale=ALPHA)

            def e2_col(h, c):
                return C_e2f[:, c, h:h + 1]

            def e2a_col(h, c):
                return C_e2a[:, c, h:h + 1]

            # --- projN: batched over heads (2 x 512-free MMs per chunk),
            # plain batched evacuation (E2 lives in s_t now)
            projAll = sproj.tile([128, NCHUNK, H * 128], BF16, tag="projAll",
                                 name=f"pa{l}", bufs=1)
            WhV = Wh_ball[l].rearrange("i h f -> i (h f)")
            for c in range(NCHUNK):
                for g in range(2):
                    pN = pmisc.tile([128, N], F32, tag="pbig")
                    nc.tensor.matmul(pN, hT_bf[:, bass.ts(c, 128)],
                                     WhV[:, bass.ts(g, 512)], start=True, stop=True)
                    dst = projAll[:, c, g * 512:(g + 1) * 512]
                    nc.scalar.activation(dst, pN, AF.Copy)

            def proj_ct(h, c):
                return projAll[:, c, h * 128:(h + 1) * 128]

            if l == 0:
                # deferred weight prep runs inside layer-0's dense phase
                prep_out_weights(0)
                prep_attn_weights(1)
                prep_out_weights(1)

            # --- attention per group of GH heads
            pous = [None] * H
            reps = [None] * H
            for g in range(NG):
                deng_ps = prow.tile([GH, N], F32, tag="deng", bufs=1)
                for j in range(GH):
                    h = g * GH + j
                    tten = sexp.tile([128, NCHUNK, N], BF16, tag="tten", bufs=3)
                    for c in range(NCHUNK):
                        nc.vector.tensor_scalar(tten[:, c, :], e1b[h],
                                                e2_col(h, c), e2a_col(h, c),
                                                OP.mult, OP.max)
                    s_t = sexp.tile([128, NCHUNK, N], BF16, tag="s_t")
                    nc.vector.tensor_tensor(s_t, tten, maskT_all, OP.mult)
                    for c in range(NCHUNK):
                        nc.tensor.matmul(deng_ps, onespad[:, j, :], s_t[:, c, :],
                                         start=(j == 0 and c == 0),
                                         stop=(j == GH - 1 and c == NCHUNK - 1))
                    pou_ps = pou.tile([128, N], F32, tag="oU")
                    for c in range(NCHUNK):
                        nc.tensor.matmul(pou_ps, proj_ct(h, c), s_t[:, c, :],
                                         start=(c == 0), stop=(c == NCHUNK - 1))
                    pob = smulti.tile([128, N], BF16, tag="pob", bufs=8)
                    nc.scalar.activation(pob, pou_ps, AF.Copy)
                    pous[h] = pob
                rr4 = srow.tile([GH, N], F32, tag="rr4")
                nc.vector.reciprocal_approx_fast(rr4, deng_ps)
                rr4_bf = srow.tile([GH, N], BF16, tag="rr4b")
                nc.vector.tensor_copy(rr4_bf, rr4)
                for j in range(GH):
                    h = g * GH + j
                    rep_ps = pmisc.tile([128, N], F32, tag="pbig")
                    nc.tensor.matmul(rep_ps, sel4[:, j, :], rr4_bf,
                                     start=True, stop=True)
                    rp = sbcast.tile([128, N], BF16, tag="rep", bufs=6)
                    nc.scalar.activation(rp, rep_ps, AF.Copy)
                    reps[h] = rp

            # --- normalize + ELU per head
            multiT = []
            for h in range(H):
                outT = sbig.tile([128, N], BF16, tag="outT", bufs=3)
                nc.vector.tensor_tensor(outT, pous[h], reps[h], OP.mult)
                ex = shd.tile([128, N], BF16, tag="elu_ex")
                nc.scalar.activation(ex, outT, AF.Exp)
                ex2 = shd.tile([128, N], BF16, tag="elu_ex2", bufs=3)
                nc.vector.tensor_scalar(ex2, ex, 1.0, -1.0, OP.min, OP.add)
                mh = smulti.tile([128, N], BF16, tag="multi")
                nc.vector.tensor_tensor(mh, outT, ex2, OP.max)
                multiT.append(mh)

            # --- W_out projection (h2 in F-layout)
            ph2 = pou.tile([128, N], F32, tag="oU")
            for h in range(H):
                nc.tensor.matmul(ph2, Wo_bf[l][:, h, :], multiT[h],
                                 start=(h == 0), stop=(h == H - 1))
            h2_bf = sbig.tile([128, N], BF16, tag="h2b", bufs=2)
            nc.scalar.activation(h2_bf, ph2, AF.Copy)

            # --- single out-attention: s12o accumulated per head from multiT
            # via aoWo (no wait on the full h2_bf)
            s12o_ps = prow.tile([2, N], F32, tag="prow")
            for h in range(H):
                nc.tensor.matmul(s12o_ps, aoWo[l][h], multiT[h],
                                 start=(h == 0), stop=(h == H - 1))
            Xo_b = sx.tile([1, N], BF16, tag="Xo_b")    # E1b_o row
            nc.scalar.activation(Xo_b, s12o_ps[0:1, :], AF.Exp, scale=BETA)
            ebo_ps = pmisc.tile([128, N], F32, tag="pbig")
            nc.tensor.matmul(ebo_ps, ones_row_bf[:, 0:128], Xo_b,
                             start=True, stop=True)
            e1bo = sbcast.tile([128, N], BF16, tag="e1b", bufs=9)
            nc.scalar.activation(e1bo, ebo_ps, AF.Copy)
            so_ps = prow.tile([128, NCHUNK, 2], F32, tag="prow")
            for c in range(NCHUNK):
                nc.tensor.matmul(so_ps[:, c, :], h2_bf[:, bass.ts(c, 128)],
                                 ao_bf[l], start=True, stop=True)
            Co_e2f = sx.tile([128, NCHUNK, 2], F32, tag="Coe2f")
            nc.scalar.activation(Co_e2f, so_ps, AF.Exp, scale=1.0)
            Co_e2a = sx.tile([128, NCHUNK, 2], F32, tag="Coe2a")
            nc.scalar.activation(Co_e2a, so_ps, AF.Exp, scale=ALPHA)
            # h2N via PE transposes of h2_bf, plain evac (E2_o lives in s_to)
            h2Np = sproj.tile([128, NCHUNK, 128], BF16, tag="h2Np")
            for c in range(NCHUNK):
                pm = pmisc.tile([128, 128], BF16, tag="pbig")
                nc.tensor.transpose(pm, h2_bf[:, bass.ts(c, 128)], ident_bf)
                nc.scalar.activation(h2Np[:, c, :], pm, AF.Copy)
            # out-att scores
            tto = sexp.tile([128, NCHUNK, N], BF16, tag="tten", bufs=3)
            for c in range(NCHUNK):
                nc.vector.tensor_scalar(tto[:, c, :], e1bo,
                                        Co_e2f[:, c, 1:2], Co_e2a[:, c, 1:2],
                                        OP.mult, OP.max)
            s_to = sexp.tile([128, NCHUNK, N], BF16, tag="s_t")
            nc.vector.tensor_tensor(s_to[:, 0:2, :], tto[:, 0:2, :],
                                    maskT_all[:, 0:2, :], OP.mult)
            nc.vector.tensor_tensor(s_to[:, 2:4, :], tto[:, 2:4, :],
                                    maskT_all[:, 2:4, :], OP.mult)
            deno_ps = prow.tile([1, N], F32, tag="prow")
            for c in range(NCHUNK):
                nc.tensor.matmul(deno_ps, ones_col_bf, s_to[:, c, :],
                                 start=(c == 0), stop=(c == NCHUNK - 1))
            pouo_ps = pou.tile([128, N], F32, tag="oU")
            for c in range(NCHUNK):
                nc.tensor.matmul(pouo_ps, h2Np[:, c, :], s_to[:, c, :],
                                 start=(c == 0), stop=(c == NCHUNK - 1))
            rro = srow.tile([1, N], F32, tag="rro")
            nc.vector.reciprocal_approx_fast(rro, deno_ps)
            rro_bf = srow.tile([1, N], BF16, tag="rrob")
            nc.vector.tensor_copy(rro_bf, rro)
            rpo_ps = pmisc.tile([128, N], F32, tag="pbig")
            nc.tensor.matmul(rpo_ps, ones_row_bf[:, 0:128], rro_bf,
                             start=True, stop=True)
            pobo = smulti.tile([128, N], BF16, tag="pobo", bufs=2)
            nc.scalar.activation(pobo, pouo_ps, AF.Copy)
            outsT = sbig.tile([128, N], BF16, tag="outsT", bufs=2)
            nc.vector.tensor_tensor(outsT, pobo, rpo_ps, OP.mult)

            # ---- residual + LN over partition dim (bf16 stream) ----
            xs = sbig.tile([128, N], BF16, tag="xs", bufs=2)
            nc.vector.tensor_tensor(xs, outsT, residT, OP.add)
            xsq = sbig.tile([128, N], BF16, tag="xsq", bufs=2)
            nc.vector.tensor_tensor(xsq, xs, xs, OP.mult)
            # 4-partition stats: row c of [4,128] = mean/meansq of chunk c
            pmu4 = prow.tile([NCHUNK, 128], F32, tag="prow")
            for c in range(NCHUNK):
                nc.tensor.matmul(pmu4, invFpad[:, c, :], xs[:, bass.ts(c, 128)],
                                 start=(c == 0), stop=(c == NCHUNK - 1))
            psq4 = prow.tile([NCHUNK, 128], F32, tag="prow")
            for c in range(NCHUNK):
                nc.tensor.matmul(psq4, invFpad[:, c, :], xsq[:, bass.ts(c, 128)],
                                 start=(c == 0), stop=(c == NCHUNK - 1))
            mu2 = srow.tile([NCHUNK, 128], F32, tag="rowL", bufs=4)
            nc.scalar.activation(mu2, pmu4, AF.Square)
            nmu4_bf = srow.tile([NCHUNK, 128], BF16, tag="rowLb", bufs=2)
            nc.vector.tensor_scalar_mul(nmu4_bf, pmu4, -1.0)
            # rstd = (var+eps)^-1/2 via int32-view seed + 2 Newton steps
            # (avoids Ln/Exp -> no ACT table switching)
            ve = srow.tile([NCHUNK, 128], F32, tag="rowL", bufs=4)
            nc.vector.scalar_tensor_tensor(ve, psq4, LN_EPS, mu2,
                                           OP.add, OP.subtract)
            y0 = srow.tile([NCHUNK, 128], F32, tag="rowL", bufs=4)
            nc.vector.tensor_scalar(y0.bitcast(mybir.dt.int32),
                                    ve.bitcast(mybir.dt.int32),
                                    -0.5, 1597463007.0, OP.mult, OP.add)
            w1 = srow.tile([NCHUNK, 128], F32, tag="rowL", bufs=4)
            nc.vector.tensor_tensor(w1, y0, y0, OP.mult)
            z1 = srow.tile([NCHUNK, 128], F32, tag="rowL", bufs=4)
            nc.vector.scalar_tensor_tensor(z1, ve, -0.5, w1, OP.mult, OP.mult)
            y1 = srow.tile([NCHUNK, 128], F32, tag="rowL", bufs=4)
            nc.vector.scalar_tensor_tensor(y1, z1, 1.5, y0, OP.add, OP.mult)
            w2 = srow.tile([NCHUNK, 128], F32, tag="rowL", bufs=4)
            nc.vector.tensor_tensor(w2, y1, y1, OP.mult)
            z2 = srow.tile([NCHUNK, 128], F32, tag="rowL", bufs=4)
            nc.vector.scalar_tensor_tensor(z2, ve, -0.5, w2, OP.mult, OP.mult)
            rstd4_bf = srow.tile([NCHUNK, 128], BF16, tag="rowLb", bufs=2)
            nc.vector.scalar_tensor_tensor(rstd4_bf, z2, 1.5, y1,
                                           OP.add, OP.mult)
            # (2 Newton steps: rstd to ~1e-5 rel; plenty for the 2e-2 gate)
            rep_rstd = pmisc.tile([128, N], F32, tag="pbig")
            rep_nmu = pmisc.tile([128, N], F32, tag="pbig")
            for c in range(NCHUNK):
                nc.tensor.matmul(rep_rstd[:, bass.ts(c, 128)],
                                 sel4[:, c, :], rstd4_bf,
                                 start=True, stop=True)
                nc.tensor.matmul(rep_nmu[:, bass.ts(c, 128)],
                                 sel4[:, c, :], nmu4_bf,
                                 start=True, stop=True)
            rep_nmu_b = sbcast.tile([128, N], BF16, tag="repl", bufs=2)
            nc.scalar.activation(rep_nmu_b, rep_nmu, AF.Copy)
            rep_rstd_b = sbcast.tile([128, N], BF16, tag="repl", bufs=2)
            nc.scalar.activation(rep_rstd_b, rep_rstd, AF.Copy)
            u = sbig.tile([128, N], BF16, tag="u", bufs=2)
            nc.vector.tensor_tensor(u, xs, rep_nmu_b, OP.add)
            t2 = sbig.tile([128, N], BF16, tag="t2", bufs=2)
            nc.vector.tensor_tensor(t2, u, rep_rstd_b, OP.mult)
            hT_bf = sbig.tile([128, N], BF16, tag="hTb", bufs=2)
            fn = AF.Relu if l < L - 1 else AF.Identity
            nc.scalar.activation(hT_bf, t2, fn, scale=g_col[l], bias=b_col[l])
            hT = hT_bf

        # ---------------- output: transpose back ----------------
        for c in range(NCHUNK):
            po = pmisc.tile([128, 128], BF16, tag="pbig")
            nc.tensor.transpose(po, hT[:, bass.ts(c, 128)], ident_bf)
            osb = shd.tile([128, 128], F32, tag="osb")
            nc.scalar.activation(osb, po, AF.Copy)
            nc.sync.dma_start(out_d[bass.ts(c, 128), :], osb)

    nc.compile()
    return nc


def _get_nc():
    if "nc" not in _CACHE:
        _CACHE["nc"] = build_nc()
    return _CACHE["nc"]


def kernel(**inputs) -> np.ndarray:
    nc = _get_nc()
    shared = {k: np.ascontiguousarray(np.asarray(inputs[k], dtype=np.float32))
              for k in ("Wp", "bp", "W_heads", "a_heads", "W_out", "a_out",
                        "ln_g", "ln_b")}
    x = np.asarray(inputs["x"], dtype=np.float32)
    adj = np.asarray(inputs["adj"], dtype=np.int32)
    in_maps = [dict(x=np.ascontiguousarray(x[b]),
                    adj=np.ascontiguousarray(adj[b]), **shared)
               for b in range(B)]
    res = run_bass_kernel_spmd(nc, in_maps, core_ids=list(range(B)))
    return np.stack([res.results[b]["out"] for b in range(B)])


if __name__ == "__main__":
    rng = np.random.default_rng(0)
    inputs = dict(
        x=rng.normal(size=(B, N, DIN)).astype(np.float32),
        adj=rng.integers(0, 2, size=(B, N, N)).astype(np.int32),
        Wp=(rng.normal(size=(DIN, F)) * 0.12).astype(np.float32),
        bp=np.zeros(F, dtype=np.float32),
        W_heads=(rng.normal(size=(L, H, F, F)) * 0.08).astype(np.float32),
        a_heads=(rng.normal(size=(L, H, 2 * F)) * 0.08).astype(np.float32),
        W_out=(rng.normal(size=(L, H * F, F)) * 0.03).astype(np.float32),
        a_out=(rng.normal(size=(L, 2 * F)) * 0.08).astype(np.float32),
        ln_g=np.ones((L, F), dtype=np.float32),
        ln_b=np.zeros((L, F), dtype=np.float32),
    )
    out = kernel(**inputs)
    print("out", out.shape, out.dtype, np.abs(out).max())
